# revision 49
# baseline (speedup 1.0000x reference)
"""Barrier_Net TRN2 kernel v2: 8-core data-parallel Bass/Tile implementation.

Structure (per core, 12800 padded agents, 25 groups of 512):
  - Layer 1 (phi/obs hidden) feature-major: 12 matmuls/group into paired
    PSUM tiles [128,1024]; relu+bias evacuation split ACT / DVE-direct /
    DMA-cast->DVE-fp16 lanes.
  - Layer 2 agent-major: deepset contraction as 48 tiny matmuls
    (out [128 agents, 16] slices), accumulated in one PSUM tile.
  - Heads: PE-transpose back to feature-major for rho1; rho2+psi1 fused
    via RP = rho_w2 @ psi_w1[:2]; psi2 agent-major into [128,8] PSUM.
  - Barrier batched over [128,1600] using ACT Square/Abs_reciprocal_sqrt
    (same act table set as Relu -> no table thrash); all tanh batched in
    a final phase (one table switch).
"""
import sys, os
sys.path.insert(0, "/opt/trn_rl_repo")
import numpy as np
import concourse.bacc as bacc
import concourse.tile as tile
import concourse.mybir as mybir
from concourse.bass_utils import run_bass_kernel_spmd
from contextlib import ExitStack

F32 = mybir.dt.float32
F16 = mybir.dt.float16
AF = mybir.ActivationFunctionType
ALU = mybir.AluOpType

B, NN, NO, SD = 100000, 16, 8, 4
H, PHI_OUT, ADIM = 64, 16, 2
DS, B_GAMMA = 0.2, 0.01
D_OBS = 85
NCORE = 8
AC = B // NCORE            # 12500 agents per core
G512 = 25                  # groups of 512
AP_ = G512 * 512           # padded agents per core = 12800
NBLK = AP_ // 128          # 100 blocks of 128 agents


def _pack_weights(phi_w1, phi_b1, phi_w2, phi_b2, obs_w1, obs_b1, obs_w2, obs_b2,
                  rho_w1, rho_b1, rho_w2, rho_b2, psi_w1, psi_b1, psi_w2, psi_b2):
    # Layer-1 lhsT: [80, 12*128]; matmul k covers elements (2k, 2k+1).
    W1L = np.zeros((81, 12 * 128), np.float32)
    for k in range(8):           # phi pairs: neighbors 2k, 2k+1
        for j in range(2):
            n = 2 * k + j
            W1L[4 * n:4 * n + 4, 128 * k + 64 * j:128 * k + 64 * j + 64] = phi_w1
    for m in range(4):           # obs pairs: obstacles 2m, 2m+1
        for j in range(2):
            o = 2 * m + j
            W1L[65 + 2 * o:65 + 2 * o + 2,
                128 * (8 + m) + 64 * j:128 * (8 + m) + 64 * j + 64] = obs_w1
    W2D = np.concatenate([phi_w2, phi_w2], 0)        # [128,16]
    OW2D = np.concatenate([obs_w2, obs_w2], 0)
    RP = rho_w2 @ psi_w1[0:2]                        # [64,64]
    GX = psi_w1[3:4]                                 # [1,64] x1 row of g
    biases = np.zeros((128, 4), np.float32)
    biases[:, 0] = np.tile(phi_b1, 2)
    biases[:, 1] = np.tile(obs_b1, 2)
    rin_bias = NN * phi_b2 + NO * obs_b2
    biases[0:64, 2] = rho_b1 + rin_bias @ rho_w1
    biases[0:64, 3] = psi_b1 + rho_b2 @ psi_w1[0:2] + float(NN) * psi_w1[2]
    return dict(W1L=W1L, W2D=W2D, OW2D=OW2D, RP=RP, GX=GX,
                R1=rho_w1, PW2=psi_w2, PB2=psi_b2, biases=biases)


def _build(nc):
    xt_d = nc.dram_tensor("xt", [81, AP_], F16, kind="ExternalInput").ap()
    xg_d = nc.dram_tensor("xg", [1, AP_], F16, kind="ExternalInput").ap()
    xbx_d = nc.dram_tensor("xbx", [128, 16 * NBLK], F32, kind="ExternalInput").ap()
    xby_d = nc.dram_tensor("xby", [128, 16 * NBLK], F32, kind="ExternalInput").ap()
    w1l_d = nc.dram_tensor("w1l", [81, 12 * 128], F16, kind="ExternalInput").ap()
    w2d_d = nc.dram_tensor("w2d", [128, 16], F16, kind="ExternalInput").ap()
    ow2d_d = nc.dram_tensor("ow2d", [128, 16], F16, kind="ExternalInput").ap()
    r1_d = nc.dram_tensor("r1", [16, 64], F16, kind="ExternalInput").ap()
    rp_d = nc.dram_tensor("rp", [64, 64], F16, kind="ExternalInput").ap()
    gx_d = nc.dram_tensor("gx", [1, 64], F16, kind="ExternalInput").ap()
    pw2_d = nc.dram_tensor("pw2", [64, 2], F16, kind="ExternalInput").ap()
    ident_d = nc.dram_tensor("ident", [128, 128], F16, kind="ExternalInput").ap()
    ones1_d = nc.dram_tensor("ones1", [1, 128], F16, kind="ExternalInput").ap()
    pb2r_d = nc.dram_tensor("pb2r", [1, 8], F16, kind="ExternalInput").ap()
    bias_d = nc.dram_tensor("biases", [128, 4], F32, kind="ExternalInput").ap()
    y_d = nc.dram_tensor("y", [128, 2 * NBLK], F32, kind="ExternalOutput").ap()

    with tile.TileContext(nc) as tc, ExitStack() as ctx:
        cw = ctx.enter_context(tc.tile_pool(name="cw", bufs=1))
        xin = ctx.enter_context(tc.tile_pool(name="xin", bufs=5))
        sp = ctx.enter_context(tc.tile_pool(name="sp", bufs=4))
        sm = ctx.enter_context(tc.tile_pool(name="sm", bufs=3))
        pp = ctx.enter_context(tc.tile_pool(name="pp", bufs=2, space="PSUM"))
        hp = ctx.enter_context(tc.tile_pool(name="hp", bufs=2, space="PSUM"))
        de = ctx.enter_context(tc.tile_pool(name="de", bufs=2, space="PSUM"))

        # ---- constants ----
        w1l = cw.tile([81, 12 * 128], F16); nc.gpsimd.dma_start(w1l[:], w1l_d)
        w2d = cw.tile([128, 16], F16); nc.gpsimd.dma_start(w2d[:], w2d_d)
        ow2d = cw.tile([128, 16], F16); nc.gpsimd.dma_start(ow2d[:], ow2d_d)
        r1t = cw.tile([16, 64], F16); nc.gpsimd.dma_start(r1t[:], r1_d)
        rpt = cw.tile([64, 64], F16); nc.gpsimd.dma_start(rpt[:], rp_d)
        gxt = cw.tile([1, 64], F16); nc.gpsimd.dma_start(gxt[:], gx_d)
        pw2t = cw.tile([64, 2], F16); nc.gpsimd.dma_start(pw2t[:], pw2_d)
        ident = cw.tile([128, 128], F16); nc.gpsimd.dma_start(ident[:], ident_d)
        ones1 = cw.tile([1, 128], F16); nc.gpsimd.dma_start(ones1[:], ones1_d)
        pb2r = cw.tile([1, 8], F16); nc.gpsimd.dma_start(pb2r[:], pb2r_d)
        biases = cw.tile([128, 4], F32); nc.gpsimd.dma_start(biases[:], bias_d)
        xbx = cw.tile([128, 16 * NBLK], F32); nc.gpsimd.dma_start(xbx[:], xbx_d)
        xby = cw.tile([128, 16 * NBLK], F32); nc.gpsimd.dma_start(xby[:], xby_d)
        # per-group [dsb(64) | e(8)] f16 copies of the d4e8 psum tile
        decw = cw.tile([128, 72 * G512], F16)
        barx = cw.tile([128, NBLK], F32)
        bary = cw.tile([128, NBLK], F32)
        # barrier work tiles (written in chunks)
        b_sq = cw.tile([128, 16 * NBLK], F32)
        b_ss = cw.tile([128, 16 * NBLK], F32)
        b_uu = cw.tile([128, 16 * NBLK], F32)
        b_vv = cw.tile([128, 16 * NBLK], F32)
        b_ww = cw.tile([128, 16 * NBLK], F32)
        b_rx = cw.tile([128, 16 * NBLK], F32)
        b_ry = cw.tile([128, 16 * NBLK], F32)

        def barrier_chunk(cs, cn):
            sl = slice(cs, cs + cn)
            nc.gpsimd.tensor_mul(b_sq[:, sl], xbx[:, sl], xbx[:, sl])
            nc.gpsimd.tensor_mul(b_ss[:, sl], xby[:, sl], xby[:, sl])
            nc.gpsimd.tensor_add(b_ss[:, sl], b_ss[:, sl], b_sq[:, sl])
            nc.scalar.activation(b_uu[:, sl], b_ss[:, sl], AF.Sqrt)
            # v = (||p|| - DS)/gamma ; r = 1/v = gamma/(||p||-DS)
            nc.gpsimd.tensor_scalar(b_vv[:, sl], b_uu[:, sl],
                                    -DS, 1.0 / B_GAMMA,
                                    op0=ALU.add, op1=ALU.mult)
            nc.vector.reciprocal_approx_fast(out=b_ww[:, sl], in_=b_vv[:, sl])
            nc.gpsimd.tensor_mul(b_rx[:, sl], b_ww[:, sl], xbx[:, sl])
            nc.gpsimd.tensor_mul(b_ry[:, sl], b_ww[:, sl], xby[:, sl])
            nb0, nb1 = cs // 16, (cs + cn) // 16
            nc.vector.tensor_reduce(
                out=barx[:, nb0:nb1],
                in_=b_rx[:, sl].rearrange("p (b n) -> p b n", n=16),
                axis=mybir.AxisListType.X, op=ALU.add)
            nc.vector.tensor_reduce(
                out=bary[:, nb0:nb1],
                in_=b_ry[:, sl].rearrange("p (b n) -> p b n", n=16),
                axis=mybir.AxisListType.X, op=ALU.add)

        NCHUNK = 10
        ccols = 16 * NBLK // NCHUNK

        def stage_A(gi):
            """Layer 1 for group gi: input DMA, 12 matmuls, relu evac to S."""
            cs = gi * 512
            xt = xin.tile([81, 512], F16, tag="xt")
            nc.gpsimd.dma_start(xt[:], xt_d[:, cs:cs + 512])
            xg = xin.tile([1, 512], F16, tag="xg")
            nc.gpsimd.dma_start(xg[:], xg_d[:, cs:cs + 512])
            S = sp.tile([128, 12 * 512], F16, tag="S")
            for i in range(6):
                pt = pp.tile([128, 1024], F32, tag="pp")
                nc.tensor.matmul(pt[:, 0:512], lhsT=w1l[:, 256 * i:256 * i + 128],
                                 rhs=xt[:, :], start=True, stop=True)
                nc.tensor.matmul(pt[:, 512:1024],
                                 lhsT=w1l[:, 256 * i + 128:256 * i + 256],
                                 rhs=xt[:, :], start=True, stop=True)
                bcol = biases[:, 0:1] if i < 4 else biases[:, 1:2]
                dst = S[:, 1024 * i:1024 * (i + 1)]
                if i in (0, 2, 3, 5):   # ACT pair lanes
                    nc.scalar.activation(dst, pt[:, :], AF.Relu, bias=bcol)
                else:                   # DVE pair lanes
                    nc.vector.tensor_scalar(dst, pt[:, :], bcol, 0.0,
                                            op0=ALU.add, op1=ALU.max)
            return S, xg

        def stage_B1(gi, S):
            # ---- layer 2 agent-major: D = deepset sum [128a, 16] per block ----
            d4e8 = de.tile([128, 72], F32, tag="d4e8")
            for c in range(4):
                for k in range(12):
                    w2k = w2d if k < 8 else ow2d
                    nc.tensor.matmul(d4e8[:, 16 * c:16 * c + 16],
                                     lhsT=S[:, 512 * k + 128 * c:512 * k + 128 * c + 128],
                                     rhs=w2k[:, :], start=(k == 0), stop=(k == 11))
            dsb = decw[:, 72 * gi:72 * gi + 64]
            nc.vector.tensor_copy(dsb, d4e8[:, 0:64])
            return d4e8, dsb

        def stage_B2(gi, d4e8, dsb, xg):
            # ---- transpose to feature-major rho input [16, 512] ----
            rt = hp.tile([16, 512], F16, tag="hps")
            for c in range(4):
                nc.tensor.transpose(rt[:, 128 * c:128 * c + 128],
                                    dsb[:, 16 * c:16 * c + 16], ident[:])
            rin = sm.tile([16, 512], F16, tag="rin")
            nc.vector.tensor_copy(rin[:], rt[:])

            # ---- rho1 + fused (rho2+psi1) + psi2 ----
            phx = hp.tile([128, 512], F32, tag="hps")
            ph = phx[0:64, :]
            ph2 = phx[64:128, :]
            nc.tensor.matmul(ph, lhsT=r1t[:, :], rhs=rin[:, :],
                             start=True, stop=True)
            rh = sm.tile([64, 512], F16, tag="rh")
            nc.vector.tensor_scalar(rh[:], ph, biases[0:64, 2:3], 0.0,
                                    op0=ALU.add, op1=ALU.max)
            nc.tensor.matmul(ph2, lhsT=rpt[:, :], rhs=rh[:, :],
                             start=True, stop=False, tile_position=(0, 64))
            nc.tensor.matmul(ph2, lhsT=gxt[:, :], rhs=xg[:, :],
                             start=False, stop=True, tile_position=(0, 64))
            psih = sm.tile([64, 512], F16, tag="psih")
            nc.vector.tensor_scalar(psih[:], ph2, biases[0:64, 3:4], 0.0,
                                    op0=ALU.add, op1=ALU.max)
            # psi2 agent-major, pb2 via init matmul
            nc.tensor.matmul(d4e8[:, 64:72], lhsT=ones1[:, :], rhs=pb2r[:, :],
                             start=True, stop=False, skip_group_check=True)
            for c in range(4):
                nc.tensor.matmul(d4e8[:, 64 + 2 * c:64 + 2 * c + 2],
                                 lhsT=psih[:, 128 * c:128 * c + 128],
                                 rhs=pw2t[:, :], start=False, stop=(c == 3),
                                 skip_group_check=True)
            nc.vector.tensor_copy(decw[:, 72 * gi + 64:72 * gi + 72],
                                  d4e8[:, 64:72])

            if gi % 2 == 0 and 1 <= gi // 2 <= NCHUNK:
                ci = gi // 2 - 1
                barrier_chunk(ci * ccols, ccols)

        # software pipeline: emit L2+dsb of group g, then layer-1 of group
        # g+2, then the head chain of group g.
        LOOKAHEAD = 2
        pend = [stage_A(g) for g in range(LOOKAHEAD)]
        for gi in range(G512):
            S, xg = pend.pop(0)
            d4e8, dsb = stage_B1(gi, S)
            if gi + LOOKAHEAD < G512:
                pend.append(stage_A(gi + LOOKAHEAD))
            stage_B2(gi, d4e8, dsb, xg)

        # ---- final phase: batched tanh + barrier add + tanh ----
        t1 = cw.tile([128, 2 * NBLK], F32)
        eview = decw[:].rearrange("p (g s) -> p g s", s=72)[:, :, 64:72]
        nc.scalar.activation(t1[:].rearrange("p (g s) -> p g s", s=8),
                             eview, AF.Tanh)
        t2 = cw.tile([128, 2 * NBLK], F32)
        t1r = t1[:].rearrange("p (b u) -> p b u", u=2)
        t2r = t2[:].rearrange("p (b u) -> p b u", u=2)
        nc.vector.tensor_add(t2r[:, :, 0:1], t1r[:, :, 0:1],
                             barx[:].rearrange("p (b o) -> p b o", o=1))
        nc.vector.tensor_add(t2r[:, :, 1:2], t1r[:, :, 1:2],
                             bary[:].rearrange("p (b o) -> p b o", o=1))
        yt = cw.tile([128, 2 * NBLK], F32)
        nc.scalar.activation(yt[:], t2[:], AF.Tanh)
        nc.sync.dma_start(y_d, yt[:])
    return nc


def _host_pack(x, wk):
    """Per-core input maps from full x [B, 85] and packed weights."""
    const = {
        "w1l": wk["W1L"].astype(np.float16),
        "w2d": wk["W2D"].astype(np.float16),
        "ow2d": wk["OW2D"].astype(np.float16),
        "r1": wk["R1"].astype(np.float16),
        "rp": wk["RP"].astype(np.float16),
        "gx": wk["GX"].astype(np.float16),
        "pw2": wk["PW2"].astype(np.float16),
        "ident": np.eye(128, dtype=np.float16),
        "ones1": np.ones((1, 128), np.float16),
        "pb2r": np.tile(wk["PB2"], 4).reshape(1, 8).astype(np.float16),
        "biases": wk["biases"].astype(np.float32),
    }
    in_maps = []
    for c in range(NCORE):
        xs = x[c * AC:(c + 1) * AC]
        xp = np.zeros((AP_, D_OBS), np.float32)
        xp[:AC] = xs
        nb = xp[:, 5:69].reshape(AP_, 16, 4)
        px = -nb[:, :, 0].copy()
        py = -nb[:, :, 1].copy()
        px[AC:] = 1.0   # pad agents: avoid rsqrt(0)
        py[AC:] = 1.0
        m = dict(const)
        xt81 = np.empty((81, AP_), np.float16)
        xt81[0:64] = xp[:, 5:69].T.astype(np.float16)
        xt81[64] = xp[:, 1].astype(np.float16)
        xt81[65:81] = xp[:, 69:85].T.astype(np.float16)
        m["xt"] = np.ascontiguousarray(xt81)
        m["xg"] = np.ascontiguousarray(xp[:, 1:2].T.astype(np.float16))
        m["xbx"] = np.ascontiguousarray(
            px.reshape(NBLK, 128, 16).transpose(1, 0, 2).reshape(128, 16 * NBLK))
        m["xby"] = np.ascontiguousarray(
            py.reshape(NBLK, 128, 16).transpose(1, 0, 2).reshape(128, 16 * NBLK))
        in_maps.append(m)
    return in_maps


_CACHED = {}


def kernel(**inputs):
    x = np.asarray(inputs["x"], np.float32)
    wk = _pack_weights(**{k: np.asarray(v, np.float32) for k, v in inputs.items()
                          if k != "x"})
    in_maps = _host_pack(x, wk)

    if "nc" not in _CACHED:
        nc = bacc.Bacc("TRN2", target_bir_lowering=False, debug=False,
                       num_devices=NCORE)
        _build(nc)
        nc.compile()
        _CACHED["nc"] = nc
    nc = _CACHED["nc"]
    trace = bool(int(os.environ.get("KERNEL_TRACE", "0")))
    res = run_bass_kernel_spmd(nc, in_maps, core_ids=list(range(NCORE)),
                               trace=trace)
    _CACHED["exec_time_ns"] = res.exec_time_ns
    _CACHED["res"] = res
    out = np.empty((B, ADIM), np.float32)
    for c in range(NCORE):
        Y = res.results[c]["y"]                      # [128, 2*NBLK]
        Y4 = 2.0 * Y.reshape(128, NBLK, 2).transpose(1, 0, 2).reshape(AP_, 2)
        out[c * AC:(c + 1) * AC] = Y4[:AC]
    return out


if __name__ == "__main__":
    import reference
    ins = {k: np.asarray(v) for k, v in reference.setup_inputs().items()}
    got = kernel(**ins)
    exp = np.asarray(reference.reference(**ins))
    err = np.abs(got - exp).max()
    rel = err / np.abs(exp).max()
    print(f"absmax {err:.4e} rel {rel:.4e}")
